# revision 35
# baseline (speedup 1.0000x reference)
"""DecoderLSTM Trainium2 kernel.

Computes, for inputs matching the reference nn module:
    x  = embed_table[captions]                      # [B, T, E]
    xg = einsum('bte,ge->tbg', x, W_ih) + b_ih + b_hh
    (h, c) LSTM scan over T steps, h0 = features, c0 = 0
    out = einsum('tbh,vh->btv', hs, W_out) + b_out  # [B, T, V]

Sharding: data-parallel over batch. 8 cores x 16 batch rows each.
Weights are replicated (cast to bf16 host-side); each core computes its
16-row slice of the output. Per-core output is produced in transposed
layout [V, T*Bc] and untransposed on the host during unshard.

Device layout notes (per core, Bc = 16 batch rows):
  - Embedding gather: per-partition indirect DMA on the gpsimd DGE
    (row idx[p,m] -> xstage[p,m,:]; no custom-ucode LOAD_LIB), then PE
    transposes into xT [128p=E-offset, m, k=E-block, 128=(t,b) col].
    Table column 383 is 1.0 so row 383 of W_ihT carries (b_ih + b_hh):
    bias folded into the xg matmul.
  - Startup DMA plan: zero DMA issues on the scalar ring (a parked
    issue blocks recurrence ACTIVATEs); weight packs go FIRST on the
    SWDGE bulk queue (tiny PE constants, then wih, whh) while the
    gathers flow in parallel on the dynamic queue; wout last (its issue
    parks ~44us for ring space). A PE warmup spin flips the HAM clock
    gate before the real matmuls.
  - Gate permutation: the 4H=2048 gate dim is reordered host-side so
    column-group j of the recurrent matmul computes
    [i_j | f_j | g_j | o_j] (H-slice j of each gate). Gates land in one
    PSUM bank as [128=(32j+b), 4, 128] and the whole nonlinearity runs
    on [128, *] tiles.
  - Recurrent matmul: single chain, full 16-row batch -- 4 concurrent
    column-tiled matmuls (col_grp 32j, M=16) stream W_hhT chunks ONCE
    per step; each group's accumulation is seeded by a shifted-identity
    matmul injecting xg straight out of xg_sb (moving window 32-aligned;
    no per-step rebase DMA).
  - h = sig(o)*tanh(c) is computed in gate space (bf16 DVE) and ONE
    PE transpose (32 live output cols via an identity-lite moving
    operand) carries it into hsT -- one transpose per tail, PE being
    the binding engine of the recurrence.
  - hsT stores slot s at pos (s-1)%21 so the projection moving operand
    hsT[:, k, 0:20, :] is contiguous (MM issue at the 136ns N=320
    floor).
  - Projection: out_T = W_out @ hs_T with W_out blocks stationary,
    V on partitions; out[:, :, t] only needs h_t, so the projection is
    TIME-CHUNKED into complete-output halves: pos 0..9 runs inside
    recurrence steps 10..19 (emitted between the gate matmuls and the
    tail transpose, filling the PE wait on the nonlinearity; evac on
    vector+gpsimd since scalar carries the acts), pos 10..19 runs after
    the recurrence (scalar+vector evac). v-tile PAIRS interleave their
    k-chunk matmuls so consecutive MMs hit different PSUM accumulation
    groups and pipeline at the stream floor (~136ns at N=320); b_out
    added during bf16 PSUM evacuation; bf16 output halves write traffic.

The single-chain step loop is latency-bound at ~(tail 2.5us + gates
1.3us) per step; the freed PE idle (vs the old two-chain scheme, which
streamed W_hh twice per step) absorbs projection chunk-1.
"""

import numpy as np
import ml_dtypes

import concourse.bass as bass
import concourse.mybir as mybir
import concourse.tile as tile
from concourse import bacc

BF16 = mybir.dt.bfloat16
F32 = mybir.dt.float32
I32 = mybir.dt.int32

B, T, E, H, V = 128, 20, 300, 512, 10000
EPAD = 384            # E padded; col 383 is the ones column (bias row)
NCORES = 8
BC = B // NCORES      # 16 batch rows per core
NT = BC * T           # 320 (t,b) columns per core
NIDX = 384            # gather idx count (padded to %128)
NV = 79               # ceil(10112 / 128) vocab row-tiles
PROJ_TILED = 0        # col-tiled projection measured slower; disabled
VPAD = NV * 128       # 10112
AF = mybir.ActivationFunctionType


def _gate_perm():
    """new gate-dim order: chunk j = [i_j | f_j | g_j | o_j], blocks of 128."""
    perm = np.empty(4 * H, dtype=np.int64)
    n = 0
    for j in range(4):
        for q in range(4):          # i, f, g, o (PyTorch LSTM order)
            for r in range(128):
                perm[n] = q * H + j * 128 + r
                n += 1
    return perm


def build_nc():
    nc = bacc.Bacc("TRN2", target_bir_lowering=False, debug=False)

    # ---- DRAM parameters (per-core shapes) ----
    emb_d = nc.dram_tensor("emb", [V, EPAD], BF16, kind="ExternalInput")
    idx_d = nc.dram_tensor("idx32", [128, 3], I32, kind="ExternalInput")
    # packC: tiny PE constants [i32sh (4s x 8) | idf 128 | idlite 32] --
    #   first on the SWDGE bulk queue so the xT transposes (idf) and inject
    #   stationaries aren't gated on the big wih transfer.
    #   i32sh[32g+r, s, b] = 1 iff r == 8*s+b (replicated per 32-group):
    #   inject stationary selecting 8 rows at sub-offset 8s of a 32-aligned
    #   moving window.
    # packA: per-partition [wih (3k x 4n x 512)]
    # packB: per-partition [whh (4k x 4j x 512)]
    # Packing weights into ONE DMA per deadline class keeps the tiny
    # (~17-entry, globally pooled) DMA-sem space free of reuse parks during
    # startup.
    packC_d = nc.dram_tensor("packC", [128, 224], BF16, kind="ExternalInput")
    packA_d = nc.dram_tensor("packA", [128, 6144], BF16, kind="ExternalInput")
    packB_d = nc.dram_tensor("packB", [128, 8192], BF16, kind="ExternalInput")
    wout_d = nc.dram_tensor("wout", [128, 4, NV, 128], BF16, kind="ExternalInput")
    bout_d = nc.dram_tensor("bout", [128, NV], F32, kind="ExternalInput")
    h0t_d = nc.dram_tensor("h0t", [128, 4, BC], BF16, kind="ExternalInput")
    outT_d = nc.dram_tensor("outT", [128, NV, NT], BF16, kind="ExternalOutput")

    HBC = 8  # live batch cols per chain in the tail transposes
    with tile.TileContext(nc) as tc:
        with (
            tc.tile_pool(name="const", bufs=1) as const,
            tc.tile_pool(name="wpool", bufs=1) as wpool,
            tc.tile_pool(name="work", bufs=2) as work,
            tc.tile_pool(name="stage", bufs=4) as stage_p,
            tc.tile_pool(name="psg", bufs=1, space="PSUM") as ps_gates,
            tc.tile_pool(name="psh", bufs=1, space="PSUM") as ps_ht,
            tc.tile_pool(name="psb", bufs=6, space="PSUM") as ps_big,
        ):
            idx_sb = const.tile([128, 3], I32, tag="idx")
            bout_sb = const.tile([128, NV], F32, tag="bout")

            # xT: [128p, m=(t,b)-chunk, k=E-block, 128=(t,b) col], e=128k+p
            xT = wpool.tile([128, 3, 3, 128], BF16, tag="xT")
            # xstage: gather landing pad, row r=128m+p of x at [p, m, :]
            xstage = wpool.tile([128, 3, EPAD], BF16, tag="xstage")
            packC = wpool.tile([128, 224], BF16, tag="packC")
            packA = wpool.tile([128, 6144], BF16, tag="packA")
            packB = wpool.tile([128, 8192], BF16, tag="packB")
            wout_sb = wpool.tile([128, 4, NV, 128], BF16, tag="wout")

            def wih_ap(k, n):
                return packA[:, k * 2048 + n * 512:k * 2048 + (n + 1) * 512]

            def i16sh_ap(base, s):
                return packC[base:base + 32, 16 * s:16 * (s + 1)]

            idf_ap = packC[:, 32:160]
            # identity-lite: only the 64 live output columns (32j+b16) of
            # the tail transpose -- shorter moving stream on the PE
            idlite_ap = packC[:, 160:224]

            def whh_ap(k, j):
                return packB[:, k * 2048 + j * 512:k * 2048 + (j + 1) * 512]
            xg_sb = [wpool.tile([128, 4, 512], BF16, tag=f"xg{m}", name=f"xg{m}") for m in range(3)]
            # hs_T: [128p=H-offset, H-block k, pos, b] with slot s (h
            # after step s-1) at pos (s-1)%21: slot 0 lives at pos 20 and
            # slots 1..20 at pos 0..19, so the projection's moving operand
            # hsT[:, k, 0:20, :] is fully contiguous.
            hsT = wpool.tile([128, 4, T + 1, BC], BF16, tag="hsT")
            Cc = wpool.tile([128, 128], F32, tag="C0", name="C0")

            gates_ps = ps_gates.tile([128, 4, 128], F32, tag="g0", name="g0")
            ht_ps = ps_ht.tile([128, 4, BC], BF16, tag="ht0", name="ht0")

            # ---- loads ----
            # sync HWDGE ring: only small latency-critical transfers (idx,
            # h0t) + the projection output writes at the end.
            # Embedding gather: per-partition indirect DMA on the gpsimd
            # DGE (row idx[p,m] -> xstage[p,m,:]), then PE transposes into
            # xT. This avoids dma_gather's custom-ucode LOAD_LIB + op
            # launch (~18us of startup in the old scheme).
            # DMA plan. Constraints learned the hard way:
            #  - scalar engine queue carries ZERO DMA issues (a parked
            #    issue blocks recurrence ACTIVATEs; measured 47us stall)
            #  - the ~17 DMA completion sems are pooled GLOBALLY in
            #    emission order; DMA #n reuses #(n-17)'s sem and its issue
            #    instruction parks until that DMA completed. So the
            #    emission order below is arranged so every reuse lands on
            #    an early-completing transfer.
            #  - the SWDGE bulk queue (q0) is FIFO (~350-430GB/s once
            #    flowing); the gathers ride a separate dynamic queue (q1),
            #    so the weight packs go FIRST on q0 (no idx dependency:
            #    packC ~4.5us, packA ~9us, packB ~15us) while the gathers
            #    issue behind them and flow in parallel on q1.
            #  - wout last: its issue instruction parks ~44us on the
            #    gpsimd queue waiting for SWDGE ring space, so nothing may
            #    be emitted on gpsimd after it.
            nc.sync.dma_start(idx_sb[:], idx_d[:])
            nc.sync.dma_start(hsT[:, :, T, :], h0t_d[:])
            nc.gpsimd.dma_start(packC[:], packC_d[:])
            nc.gpsimd.dma_start(packA[:], packA_d[:])
            nc.gpsimd.dma_start(packB[:], packB_d[:])
            for m in range(3):
                nc.gpsimd.indirect_dma_start(
                    out=xstage[:, m, :], out_offset=None,
                    in_=emb_d[:],
                    in_offset=bass.IndirectOffsetOnAxis(
                        ap=idx_sb[:, m:m + 1], axis=0))
            nc.gpsimd.dma_start(bout_sb[:], bout_d[:])
            # wout completes ~60us, well before the projection needs it
            nc.gpsimd.dma_start(wout_sb[:], wout_d[:])
            nc.vector.memset(gates_ps[:], 0.0)
            nc.vector.memset(Cc[:], 0.0)

            # PE warmup: ~6us of dummy matmuls while the gathers/weights
            # are still in flight flips the HAM clock gate to 8/8 before
            # the real startup matmuls issue (otherwise the xT transposes
            # + xg matmuls all run at the cold 1.2GHz rate)
            warm = work.tile([128, 512], BF16, tag="warm")
            nc.vector.memset(warm[:], 0.0)
            # pre-warm the ACT function tables (Sigmoid/Tanh for the tails,
            # Identity for the projection evacs) in startup dead time so
            # the first real activation doesn't pay the ~1.3us table load
            # on the critical path
            actwarm = work.tile([128, 1], F32, tag="actwarm")
            nc.scalar.activation(actwarm[:], warm[:, 0:1], AF.Sigmoid)
            nc.scalar.activation(actwarm[:], warm[:, 0:1], AF.Tanh)
            nc.scalar.activation(actwarm[:], warm[:, 0:1], AF.Identity)
            for _ in range(12):
                wp = ps_big.tile([128, 512], F32, tag="big")
                nc.tensor.matmul(wp[:], warm[:, 0:128], warm[:],
                                 start=True, stop=True)

            # xstage -> xT via PE transposes (PE is idle at startup; a DMA
            # transpose here would add 3 more DMAs to the startup sem pool
            # and re-introduce reuse parks)
            def emit_xT(m):
                for k in range(3):
                    tp = ps_big.tile([128, 128], BF16, tag="big",
                                     name=f"xtp{m}_{k}")
                    nc.tensor.transpose(
                        tp[:], xstage[:, m, 128 * k:128 * (k + 1)], idf_ap)
                    nc.vector.tensor_copy(xT[:, m, k, :], tp[:])

            def emit_xT_half(m, lo):
                """Transpose (t,b) rows lo..lo+64 of m-tile m only. The
                identity slice idf[lo:lo+64, lo:lo+64] is I64 with matching
                base partition."""
                for k in range(3):
                    tp = ps_big.tile([128, 64], BF16, tag="big",
                                     name=f"xtp{m}_{k}_{lo}")
                    nc.tensor.transpose(
                        tp[:], xstage[lo:lo + 64, m, 128 * k:128 * (k + 1)],
                        idf_ap[lo:lo + 64, lo:lo + 64])
                    nc.vector.tensor_copy(xT[:, m, k, lo:lo + 64], tp[:])

            # ---- xg = x @ W_ihT -> [(t,b) rows, 2048 perm'd gate cols] ----
            # m-tile 0 runs as two 64-row halves: the lo half (steps 0-3)
            # only needs the 64-row mini-gather + packA, so the recurrence
            # starts ~12us earlier; the hi half (steps 4-7) + m1/m2 are
            # emitted inside the step loop as PE filler.
            def emit_xg(m, n):
                ps = ps_big.tile([128, 512], F32, tag="big")
                for k in range(3):
                    nc.tensor.matmul(
                        ps[:],
                        xT[:, m, k, :],
                        wih_ap(k, n),
                        start=(k == 0), stop=(k == 2),
                    )
                # evacuate on DVE only: the ACT queue carries the weight-DMA
                # issue instructions at kernel start, which would delay xg
                nc.vector.tensor_copy(xg_sb[m][:, n, :], ps[:])

            def emit_xg_half(m, n, lo):
                ps = ps_big.tile([128, 512], F32, tag="big",
                                 name=f"xgh{m}_{n}_{lo}")
                for k in range(3):
                    nc.tensor.matmul(
                        ps[lo:lo + 64, :],
                        xT[:, m, k, lo:lo + 64],
                        wih_ap(k, n),
                        start=(k == 0), stop=(k == 2),
                    )
                nc.vector.tensor_copy(xg_sb[m][lo:lo + 64, n, :],
                                      ps[lo:lo + 64, :])

            emit_xT(0)
            for n in range(4):
                emit_xg(0, n)
            emit_xT(1)
            emit_xT(2)

            def emit_inject(t):
                """xg seed for step t's gates PSUM. Depends only on the
                precomputed xg (not on h), so it is emitted DURING step
                t-1's tail wait: the WAR dep on the previous acts' reads
                of gates_ps orders it safely, and the PE streams it in
                otherwise-idle time, shrinking the serial gates block
                from 20 to 16 matmuls. Moving base must be 32-aligned:
                stream the aligned 32-row window and select rows
                r0..r0+16 via the shifted-identity stationary."""
                gp = gates_ps
                m, r0 = t // 8, (t % 8) * BC
                base, s = r0 & ~31, (r0 % 32) // 16
                for j in range(4):
                    nc.tensor.matmul(
                        gp[32 * j:32 * j + BC, :, :],
                        i16sh_ap(base, s),
                        xg_sb[m][base:base + 32, j, :],
                        start=True, stop=False,
                        tile_position=(base, 32 * j),
                        skip_group_check=True,
                    )

            def emit_gates(t):
                """Recurrent gate matmuls for step t (accumulate onto the
                inject seed): ONE whh stream per step, 4-way col-tiled."""
                gp = gates_ps
                for k in range(4):
                    for j in range(4):
                        nc.tensor.matmul(
                            gp[32 * j:32 * j + BC, :, :],
                            hsT[:, k, (t - 1) % (T + 1), 0:BC],
                            whh_ap(k, j),
                            start=False, stop=(k == 3),
                            tile_position=(0, 32 * j),
                            skip_group_check=True,
                        )

            def emit_tail(t, after_acts=None):
                """Nonlinearity for step t (all 16 rows at once: acts cost
                by free size, so one tail serves the whole batch).
                h = sig(o)*tanh(c) in gate space (bf16 DVE), ONE PE
                transpose into hsT. after_acts is emitted right after the
                three gate-reading activations (the LAST readers of
                gates_ps) so a hoisted inject for step t+1 picks up its
                WAR dependency on them."""
                gp, tp = gates_ps, ht_ps
                A = work.tile([128, 4, 128], F32, tag="A0", name=f"A_{t}")
                Ao = work.tile([128, 128], BF16, tag="Ao0", name=f"Ao_{t}")
                nc.scalar.activation(A[:, 0:2, :], gp[:, 0:2, :], AF.Sigmoid)
                nc.scalar.activation(A[:, 2, :], gp[:, 2, :], AF.Tanh)
                nc.scalar.activation(Ao[:], gp[:, 3, :], AF.Sigmoid)
                if after_acts is not None:
                    after_acts()
                T2 = work.tile([128, 128], F32, tag="T20", name=f"T2_{t}")
                T1 = work.tile([128, 128], F32, tag="T10", name=f"T1_{t}")
                TC = work.tile([128, 128], BF16, tag="TC0", name=f"TC_{t}")
                Hb = work.tile([128, 128], BF16, tag="Hb0", name=f"Hb_{t}")
                nc.vector.tensor_mul(T2[:], A[:, 0, :], A[:, 2, :])   # i*g
                nc.vector.tensor_mul(T1[:], A[:, 1, :], Cc[:])        # f*c
                nc.vector.tensor_add(Cc[:], T1[:], T2[:])
                nc.scalar.activation(TC[:], Cc[:], AF.Tanh)
                nc.vector.tensor_mul(Hb[:], Ao[:], TC[:])             # h
                nc.tensor.transpose(tp[:], Hb[:], idlite_ap)
                nc.vector.tensor_copy(hsT[:, :, t, 0:BC], tp[:])

            # ---- projection, time-chunked ----
            # out[:, :, t] only needs h_t, so the projection splits into
            # complete-output time chunks (no partial sums). v-tiles run
            # in PAIRS with k-chunks interleaved (136ns/MM stream floor).
            # Chunk-1 (pos 0..9) is emitted INSIDE recurrence steps 10..19
            # between the gate MMs and the tail transpose, filling the PE
            # wait on the nonlinearity; its PSUM evac runs on vector+gpsimd
            # (scalar carries the recurrence acts). Chunk-2 (pos 10..19)
            # runs after the recurrence on scalar+vector.
            def make_proj(p0, p1):
                """Time-chunk projection pos p0..p1. emit_mms() queues the
                PE work for n pairs (keeping PSUM tiles pending);
                flush() emits the evac+DMA for all pending pairs. The
                split lets the recurrence emit chunk MMs in the PE window
                before the tail transpose while evacs land on scalar/
                vector AFTER the tail's critical acts/muls each step."""
                cn = (p1 - p0) * BC
                c0 = p0 * BC
                state = {"v": 0, "st": None, "pend": []}

                def emit_mms(npairs):
                    for _ in range(npairs):
                        v = state["v"]
                        if v >= NV:
                            return
                        vs = [v] if v == NV - 1 else [v, v + 1]
                        pps = [ps_big.tile([128, cn], F32, tag="big",
                                           name=f"pj{p0}_{vv}")
                               for vv in vs]
                        for k in range(4):
                            for i in range(len(vs)):
                                nc.tensor.matmul(
                                    pps[i][:],
                                    wout_sb[:, k, vs[i], :],
                                    hsT[:, k, p0:p1, :],
                                    start=(k == 0), stop=(k == 3),
                                )
                        for i, vv in enumerate(vs):
                            state["pend"].append((vv, pps[i]))
                        state["v"] = v + len(vs)

                def flush():
                    for vv, pp in state["pend"]:
                        g = vv % 4
                        if g == 0:
                            nv = min(4, NV - vv)
                            state["st"] = stage_p.tile(
                                [128, nv, cn], BF16, tag="st",
                                name=f"st{p0}_{vv}")
                        st = state["st"]
                        if vv % 2 == 0:
                            nc.scalar.activation(
                                st[:, g, :], pp[:], AF.Identity,
                                bias=bout_sb[:, vv:vv + 1])
                        else:
                            nc.vector.tensor_scalar_add(
                                st[:, g, :], pp[:], bout_sb[:, vv:vv + 1])
                        if g == 3 or vv == NV - 1:
                            v0 = (vv // 4) * 4
                            nc.sync.dma_start(
                                outT_d[:, v0:vv + 1, c0:c0 + cn], st[:])
                    state["pend"] = []

                return emit_mms, flush, state

            proj1_mms, proj1_flush, proj1_state = make_proj(0, 10)
            proj2_mms, proj2_flush, proj2_state = make_proj(10, T)

            # ---- recurrence: single chain, full 16-row batch ----
            # Emission per step: MM(t), chunk-1 projection pairs (they fill
            # the PE FIFO slot where it would otherwise idle waiting for
            # the tail's Hb), then the tail (transpose last).
            emit_inject(0)
            for t in range(T):
                emit_gates(t)
                if t >= 10:
                    proj1_mms(3)
                emit_tail(t, after_acts=(
                    (lambda tt=t: emit_inject(tt + 1))
                    if t + 1 < T else None))
                if t >= 10:
                    proj1_flush()
                # PE filler during the act windows: xg m1/m2 groups
                if t < 4:
                    emit_xg(1, t)
                elif t < 8:
                    emit_xg(2, t - 4)
            # chunk-1 leftovers
            while proj1_state["v"] < NV:
                proj1_mms(2)
                proj1_flush()

            # ---- chunk-2: pos 10..19 for all v-tiles ----
            while proj2_state["v"] < NV:
                proj2_mms(2)
                proj2_flush()

    nc.compile()
    return nc


def prep_inputs(features, captions, embed_table, W_ih, W_hh, b_ih, b_hh,
                W_out, b_out):
    """Host-side shard + layout prep. Returns per-core input maps."""
    bf = ml_dtypes.bfloat16
    features = np.asarray(features, dtype=np.float32)
    captions = np.asarray(captions).astype(np.int64)
    embed_table = np.asarray(embed_table, dtype=np.float32)
    W_ih = np.asarray(W_ih, dtype=np.float32)
    W_hh = np.asarray(W_hh, dtype=np.float32)
    b_ih = np.asarray(b_ih, dtype=np.float32)
    b_hh = np.asarray(b_hh, dtype=np.float32)
    W_out = np.asarray(W_out, dtype=np.float32)
    b_out = np.asarray(b_out, dtype=np.float32)

    perm = _gate_perm()

    emb = np.zeros((V, EPAD), dtype=bf)
    emb[:, :E] = embed_table.astype(bf)
    emb[:, EPAD - 1] = bf(1.0)

    wih = np.zeros((EPAD, 4 * H), dtype=np.float32)
    wih[:E, :] = W_ih.T[:, perm]
    wih[EPAD - 1, :] = (b_ih + b_hh)[perm]
    wih = wih.astype(bf).reshape(3, 128, 4, 512)

    whh = np.ascontiguousarray(W_hh.T[:, perm]).astype(bf).reshape(4, 128, 4, 512)

    wout = np.zeros((H, VPAD), dtype=np.float32)
    wout[:, :V] = W_out.T
    wout = wout.astype(bf).reshape(4, 128, NV, 128)

    boutp = np.zeros((VPAD,), dtype=np.float32)
    boutp[:V] = b_out
    bout_r = np.ascontiguousarray(boutp.reshape(NV, 128).T)

    idf = np.eye(128, dtype=bf)
    i16sh = np.zeros((128, 2, 16), dtype=bf)
    for g in range(4):
        for s in range(2):
            for b in range(16):
                i16sh[32 * g + 16 * s + b, s, b] = bf(1.0)

    idlite = np.zeros((128, 64), dtype=bf)
    for cc in range(64):
        idlite[32 * (cc // 16) + cc % 16, cc] = bf(1.0)
    packC = np.concatenate([i16sh.reshape(128, 32), idf, idlite], axis=1)
    packA = np.ascontiguousarray(
        wih.transpose(1, 0, 2, 3)).reshape(128, 6144)
    packB = np.ascontiguousarray(whh.transpose(1, 0, 2, 3)).reshape(128, 8192)
    wout_r = np.ascontiguousarray(wout.transpose(1, 0, 2, 3))

    shared = dict(emb=emb, packC=packC, packA=packA, packB=packB,
                  wout=wout_r, bout=bout_r)

    in_maps = []
    for c in range(NCORES):
        cap_c = captions[c * BC:(c + 1) * BC]                 # [16, 20]
        # per-partition gather indices: xstage row r=128m+p holds
        # x[(t=r//16, b=r%16)]; rows >= 320 gather row 0 (unused)
        r = np.arange(3 * 128)
        flat = np.where(r < NT, cap_c[np.minimum(r % BC, BC - 1),
                                      np.minimum(r // BC, T - 1)], 0)
        idx32 = np.ascontiguousarray(
            flat.reshape(3, 128).T.astype(np.int32))          # [128, 3]
        feat_c = features[c * BC:(c + 1) * BC]                # [16, 512]
        h0t = np.ascontiguousarray(
            feat_c.reshape(BC, 4, 128).transpose(2, 1, 0)).astype(bf)
        in_maps.append(dict(shared, idx32=idx32, h0t=h0t))
    return in_maps


def unshard_one(o):
    """[128, NV, NT] bf16 -> [BC, T, V] f32 for one core."""
    o = np.asarray(o, dtype=np.float32)              # [128, NV, NT]
    o = o.transpose(1, 0, 2).reshape(VPAD, NT)[:V]                 # [V, 320]
    return o.reshape(V, T, BC).transpose(2, 1, 0)                  # [16, T, V]


def unshard(core_outs):
    """core_outs: list of 8 arrays [128, NV, NT] bf16 -> full [B, T, V] f32."""
    return np.ascontiguousarray(
        np.concatenate([unshard_one(o) for o in core_outs], axis=0))


_NC_CACHE = {}


def kernel(**inputs) -> np.ndarray:
    from concourse.bass_utils import run_bass_kernel_spmd

    if "nc" not in _NC_CACHE:
        _NC_CACHE["nc"] = build_nc()
    nc = _NC_CACHE["nc"]

    in_maps = prep_inputs(**inputs)
    res = run_bass_kernel_spmd(nc, in_maps, core_ids=list(range(NCORES)))
    return unshard([res.results[c]["outT"] for c in range(NCORES)])

